# revision 12
# baseline (speedup 1.0000x reference)
"""DGCN hypernetwork GNN kernel for 8x Trainium2 NeuronCores.

Strategy:
  Kernel 1 (data-parallel over batch, 2 samples/core):
    hypernet MLP -> nodevec V^T; per sample: A = V V^T emitted tile-by-tile on
    the PE (2-way row-group packing since E=16), relu+rowsum fused into the
    PSUM->SBUF eviction (vector tensor_scalar / scalar activation, both with
    accum_out), d = rsqrt(rowsum), z = relu(A) @ (d*x).  relu(A) lives only in
    SBUF (16 MB/sample) - never touches HBM.
  Host: y = d*z, assemble x_g^T = [x^T; y^T], reshard by node.
  Kernel 2 (data-parallel over nodes, 256 nodes/core):
    W[n] = sum_d emb1[n,d] pool[d] materialized on PE, block-diagonal
    projection out[:,n,:] = xg[:,n,:] @ W[n] + bias[n].
"""

import numpy as np

# ---------------------------------------------------------------- shapes
B, N, C, E, O = 16, 2048, 64, 16, 64
H, M, K = 16, 2, 2
NCORES = 8
BS = B // NCORES          # samples per core in kernel 1
NS = N // NCORES          # nodes per core in kernel 2
BN = BS * N               # 4096 rows per core in kernel 1
NCH = N // 128            # 16 m-chunks per sample
KI = K * C                # 128


# ------------------------------------------------- walrus drain workaround
def _apply_tile_patch():
    """This walrus build lowers at most ONE sync wait per CTRL instruction;
    Tile's end-of-kernel drain carries several.  Split extras onto Nops."""
    import concourse.mybir as mybir
    from concourse import tile

    if getattr(tile.TileContext, "_drain_split_patched", False):
        return
    orig = tile.TileContext._drain_and_barrier

    def _split_multiwait(nc):
        for f in nc.m.functions:
            for bb in f.blocks:
                newlist = []
                changed = False
                for ins in bb.instructions:
                    si = ins.sync_info
                    if si is not None and si.on_wait and len(si.on_wait) > 1:
                        waits = list(si.on_wait)
                        for w in waits[:-1]:
                            nop = mybir.InstNoOp(
                                name=f"I-{nc.next_id()}", ins=[], outs=[])
                            nop.engine = ins.engine
                            nop.sync_info = mybir.SyncInfo(
                                on_wait=[w], on_update=[])
                            nc.register_instruction(nop)
                            newlist.append(nop)
                        ins.sync_info = mybir.SyncInfo(
                            on_wait=[waits[-1]], on_update=si.on_update)
                        changed = True
                    newlist.append(ins)
                if changed:
                    bb.instructions[:] = newlist

    def patched(self, tick_clock, wait_clock):
        orig(self, tick_clock, wait_clock)
        _split_multiwait(self.nc)

    tile.TileContext._drain_and_barrier = patched
    tile.TileContext._drain_split_patched = True


# ---------------------------------------------------------------- kernel 1
def _build_k1():
    from concourse import bass, tile
    import concourse.mybir as mybir

    dt = mybir.dt
    f32 = dt.float32
    nc = bass.Bass()

    xr = nc.dram_tensor("xr", [BS, 128, NCH * C], f32, kind="ExternalInput").ap()
    xT = nc.dram_tensor("xT", [C, BN], f32, kind="ExternalInput").ap()
    e0T = nc.dram_tensor("e0T", [E, BN], f32, kind="ExternalInput").ap()
    w1 = nc.dram_tensor("w1", [C, H], f32, kind="ExternalInput").ap()
    b1 = nc.dram_tensor("b1", [H, 1], f32, kind="ExternalInput").ap()
    w2 = nc.dram_tensor("w2", [H, M], f32, kind="ExternalInput").ap()
    b2 = nc.dram_tensor("b2", [M, 1], f32, kind="ExternalInput").ap()
    w3 = nc.dram_tensor("w3", [M, E], f32, kind="ExternalInput").ap()
    b3 = nc.dram_tensor("b3", [E, 1], f32, kind="ExternalInput").ap()
    zT_out = nc.dram_tensor("zT", [BS, C, N], f32, kind="ExternalOutput").ap()
    d_out = nc.dram_tensor("dcol", [BS, 128, NCH], f32, kind="ExternalOutput").ap()

    AF = mybir.ActivationFunctionType
    AL = mybir.AluOpType

    from contextlib import ExitStack
    with tile.TileContext(nc) as tc, ExitStack() as ctx:
        cpool = ctx.enter_context(tc.tile_pool(name="consts", bufs=1))
        w1_s = cpool.tile([C, H], f32, tag="w1")
        nc.sync.dma_start(w1_s[:], w1[:])
        w2_s = cpool.tile([H, M], f32, tag="w2")
        nc.sync.dma_start(w2_s[:], w2[:])
        w3_s = cpool.tile([M, E], f32, tag="w3")
        nc.sync.dma_start(w3_s[:], w3[:])
        b1_s = cpool.tile([H, 1], f32, tag="b1")
        nc.sync.dma_start(b1_s[:], b1[:])
        b2_s = cpool.tile([M, 1], f32, tag="b2")
        nc.sync.dma_start(b2_s[:], b2[:])
        b3_s = cpool.tile([E, 1], f32, tag="b3")
        nc.sync.dma_start(b3_s[:], b3[:])

        big = ctx.enter_context(tc.tile_pool(name="big", bufs=1))
        # relu(A) store for one sample: 16 chunk-rows of [128, 2048]
        Tbig = big.tile([128, NCH * N], f32, tag="Tbig")
        # V^T replicated at partition offsets 0 and 32, one per sample
        vrep = [big.tile([48, N], f32, tag=f"vrep{s}", name=f"vrep{s}") for s in range(BS)]
        # x in [m-chunk partition, (chunk, c)] layout, per sample
        xs = [big.tile([128, NCH * C], f32, tag=f"xs{s}", name=f"xs{s}") for s in range(BS)]
        xp = big.tile([128, NCH * C], f32, tag="xp")
        zTs = big.tile([C, N], f32, tag="zTs")
        acc = big.tile([128, 2 * NCH], f32, tag="acc")
        rcol = big.tile([128, NCH], f32, tag="rcol")
        rinv = big.tile([128, NCH], f32, tag="rinv")
        dcol = big.tile([128, NCH], f32, tag="dcol")

        for s in range(BS):
            nc.sync.dma_start(xs[s][:], xr[s])

        # ---------------- hypernet MLP, feature-on-partition layout -------
        CH = 512
        with tc.tile_pool(name="mlp", bufs=2) as mp, \
             tc.tile_pool(name="mlppsum", bufs=2, space="PSUM") as pp:
            for ch in range(BN // CH):
                s, off = divmod(ch * CH, N)
                xTc = mp.tile([C, CH], f32, tag="xTc")
                nc.sync.dma_start(xTc[:], xT[:, bass.ts(ch, CH)])
                e0c = mp.tile([E, CH], f32, tag="e0c")
                nc.sync.dma_start(e0c[:], e0T[:, bass.ts(ch, CH)])

                p1 = pp.tile([H, CH], f32, tag="p1")
                nc.tensor.matmul(p1[:], lhsT=w1_s[:], rhs=xTc[:],
                                 start=True, stop=True)
                h1 = mp.tile([H, CH], f32, tag="h1")
                nc.scalar.activation(h1[:], p1[:], AF.Sigmoid, bias=b1_s[:])

                p2 = pp.tile([M, CH], f32, tag="p2")
                nc.tensor.matmul(p2[:], lhsT=w2_s[:], rhs=h1[:],
                                 start=True, stop=True)
                h2 = mp.tile([M, CH], f32, tag="h2")
                nc.scalar.activation(h2[:], p2[:], AF.Sigmoid, bias=b2_s[:])

                p3 = pp.tile([E, CH], f32, tag="p3")
                nc.tensor.matmul(p3[:], lhsT=w3_s[:], rhs=h2[:],
                                 start=True, stop=True)
                filt = mp.tile([E, CH], f32, tag="filt")
                nc.scalar.activation(filt[:], p3[:], AF.Identity, bias=b3_s[:])
                # nodevec = tanh(emb0 * filt)
                prod = mp.tile([E, CH], f32, tag="prod")
                nc.vector.tensor_tensor(out=prod[:], in0=filt[:], in1=e0c[:],
                                        op=AL.mult)
                nc.scalar.activation(vrep[s][0:E, off:off + CH], prod[:],
                                     AF.Tanh)
        for s in range(BS):
            nc.sync.dma_start(vrep[s][32:32 + E, :], vrep[s][0:E, :])

        # ---------------- per-sample adjacency + propagate ----------------
        for s in range(BS):
            # emit A = V V^T chunk-row by chunk-row; relu+rowsum on eviction
            with tc.tile_pool(name=f"pa{s}", bufs=2, space="PSUM") as pa_pool:
                for i in range(NCH):
                    g = 32 * (i % 2)
                    lhs = vrep[s][g:g + E, bass.ts(i, 128)]
                    paL = pa_pool.tile([128, 1024], f32, tag="paL")
                    paR = pa_pool.tile([128, 1024], f32, tag="paR")
                    for q in range(2):
                        nc.tensor.matmul(
                            paL[:, bass.ts(q, 512)], lhsT=lhs,
                            rhs=vrep[s][g:g + E, bass.ts(q, 512)],
                            start=True, stop=True, tile_position=(g, 0))
                    for q in range(2):
                        nc.tensor.matmul(
                            paR[:, bass.ts(q, 512)], lhsT=lhs,
                            rhs=vrep[s][g:g + E, bass.ts(2 + q, 512)],
                            start=True, stop=True, tile_position=(g, 0))
                    nc.vector.tensor_scalar(
                        Tbig[:, i * N:i * N + 1024], paL[:], 0.0, None,
                        op0=AL.max, op1=AL.add, accum_out=acc[:, i:i + 1])
                    nc.scalar.activation(
                        Tbig[:, i * N + 1024:(i + 1) * N], paR[:], AF.Relu,
                        accum_out=acc[:, NCH + i:NCH + i + 1])

            # d = 1/sqrt(rowsum)
            nc.vector.tensor_tensor(out=rcol[:], in0=acc[:, 0:NCH],
                                    in1=acc[:, NCH:2 * NCH], op=AL.add)
            nc.vector.reciprocal(rinv[:], rcol[:])
            nc.scalar.activation(dcol[:], rinv[:], AF.Sqrt)
            nc.sync.dma_start(d_out[s], dcol[:])

            # x' = d * x   (split across vector/scalar engines)
            for c in range(NCH):
                if c % 2 == 0:
                    nc.vector.tensor_scalar(
                        xp[:, bass.ts(c, C)], xs[s][:, bass.ts(c, C)],
                        dcol[:, c:c + 1], None, op0=AL.mult)
                else:
                    nc.scalar.activation(
                        xp[:, bass.ts(c, C)], xs[s][:, bass.ts(c, C)],
                        AF.Copy, scale=dcol[:, c:c + 1])

            # z^T = (relu(A) @ x')^T  via x' stationary, A moving
            with tc.tile_pool(name=f"pz{s}", bufs=1, space="PSUM") as pz_pool:
                pz = pz_pool.tile([C, N], f32, tag="pz")
                for j in range(4):
                    for c in range(NCH):
                        nc.tensor.matmul(
                            pz[:, bass.ts(j, 512)],
                            lhsT=xp[:, bass.ts(c, C)],
                            rhs=Tbig[:, c * N + 512 * j:c * N + 512 * (j + 1)],
                            start=(c == 0), stop=(c == NCH - 1))
                nc.vector.tensor_copy(zTs[:, 0:1024], pz[:, 0:1024])
                nc.scalar.copy(zTs[:, 1024:2048], pz[:, 1024:2048])
            nc.sync.dma_start(zT_out[s], zTs[:])

    return nc


# ---------------------------------------------------------------- kernel 2
def _build_k2():
    from concourse import bass, tile
    import concourse.mybir as mybir

    dt = mybir.dt
    f32 = dt.float32
    nc = bass.Bass()

    e1T = nc.dram_tensor("e1T", [E, NS], f32, kind="ExternalInput").ap()
    poolT = nc.dram_tensor("poolT", [E, O * KI], f32, kind="ExternalInput").ap()
    bp = nc.dram_tensor("bp", [E, O], f32, kind="ExternalInput").ap()
    xgTs = nc.dram_tensor("xgTs", [KI, NS * B], f32, kind="ExternalInput").ap()
    outT = nc.dram_tensor("outT", [O, NS * B], f32, kind="ExternalOutput").ap()

    AL = mybir.AluOpType

    with tile.TileContext(nc) as tc:
        with tc.tile_pool(name="sb", bufs=1) as sb, \
             tc.tile_pool(name="pw", bufs=3, space="PSUM") as pwp, \
             tc.tile_pool(name="po", bufs=4, space="PSUM") as pop:
            e1T_s = sb.tile([E, NS], f32, tag="e1T")
            nc.sync.dma_start(e1T_s[:], e1T[:])
            pT_s = sb.tile([E, O * KI], f32, tag="pT")
            nc.sync.dma_start(pT_s[:], poolT[:])
            bp_s = sb.tile([E, O], f32, tag="bp")
            nc.sync.dma_start(bp_s[:], bp[:])
            xg_s = sb.tile([KI, NS * B], f32, tag="xg")
            nc.sync.dma_start(xg_s[:], xgTs[:])
            Ws = sb.tile([KI, O * NS], f32, tag="Ws")
            biasT = sb.tile([O, NS], f32, tag="biasT")
            outs = sb.tile([O, NS * B], f32, tag="outs")

            # W[:, o*NS + n] over ki partitions = sum_d emb1[n,d] pool[d,ki,o]
            for o in range(O):
                pw = pwp.tile([KI, NS], f32, tag="pw")
                nc.tensor.matmul(pw[:], lhsT=pT_s[:, bass.ts(o, KI)],
                                 rhs=e1T_s[:], start=True, stop=True)
                if o % 2 == 0:
                    nc.vector.tensor_copy(Ws[:, bass.ts(o, NS)], pw[:])
                else:
                    nc.scalar.copy(Ws[:, bass.ts(o, NS)], pw[:])

            pb = pwp.tile([O, NS], f32, tag="pb", bufs=1)
            nc.tensor.matmul(pb[:], lhsT=bp_s[:], rhs=e1T_s[:],
                             start=True, stop=True)
            nc.vector.tensor_copy(biasT[:], pb[:])

            # per-node projection, 8 nodes per PSUM bank group
            Wv = Ws[:].rearrange("p (o n) -> p o n", o=O, n=NS)
            for grp in range(NS // 8):
                po = pop.tile([O, 8 * B], f32, tag="po")
                for t in range(8):
                    n = 8 * grp + t
                    nc.tensor.matmul(
                        po[:, bass.ts(t, B)], lhsT=Wv[:, :, n:n + 1],
                        rhs=xg_s[:, bass.ts(n, B)], start=True, stop=True)
                bslice = biasT[:, 8 * grp:8 * grp + 8]
                bbc = bslice.unsqueeze(2).broadcast_to([O, 8, B])
                nc.vector.tensor_tensor(
                    out=outs[:, grp * 8 * B:(grp + 1) * 8 * B], in0=po[:],
                    in1=bbc, op=AL.add)
            nc.sync.dma_start(outT[:], outs[:])
    return nc


_PROGRAMS = {}
_LAST_RESULTS = []
_LAST_WALL = []


def _programs():
    if "k1" not in _PROGRAMS:
        _apply_tile_patch()
        _PROGRAMS["k1"] = _build_k1()
        _PROGRAMS["k2"] = _build_k2()
    return _PROGRAMS["k1"], _PROGRAMS["k2"]


class _Runner:
    """Cached jitted SPMD executor (mirrors bass2jax.run_bass_via_pjrt but
    keeps the jit closure alive so repeat calls don't recompile)."""

    def __init__(self, nc):
        import jax
        import concourse.mybir as mybir
        from jax.sharding import Mesh, PartitionSpec
        from jax.experimental.shard_map import shard_map
        from concourse.bass2jax import (
            _bass_exec_p, install_neuronx_cc_hook, partition_id_tensor)

        install_neuronx_cc_hook()
        self.nc = nc
        part_name = (nc.partition_id_tensor.name
                     if nc.partition_id_tensor else None)
        in_names, out_names, out_avals, zero_shapes = [], [], [], []
        for alloc in nc.m.functions[0].allocations:
            if not isinstance(alloc, mybir.MemoryLocationSet):
                continue
            name = alloc.memorylocations[0].name
            if alloc.kind == "ExternalInput":
                if name != part_name:
                    in_names.append(name)
            elif alloc.kind == "ExternalOutput":
                out_names.append(name)
                shape = tuple(alloc.tensor_shape)
                dtype = mybir.dt.np(alloc.dtype)
                out_avals.append(jax.core.ShapedArray(shape, dtype))
                zero_shapes.append((shape, dtype))
        self.in_names, self.out_names = in_names, out_names
        self.out_avals, self.zero_shapes = out_avals, zero_shapes
        n_params = len(in_names)
        all_names = tuple(in_names + out_names
                          + ([part_name] if part_name else []))
        donate = tuple(range(n_params, n_params + len(out_names)))

        def _body(*args):
            operands = list(args)
            if part_name is not None:
                operands.append(partition_id_tensor())
            outs = _bass_exec_p.bind(
                *operands, out_avals=tuple(out_avals), in_names=all_names,
                out_names=tuple(out_names),
                lowering_input_output_aliases=(),
                sim_require_finite=True, sim_require_nnan=True, nc=nc)
            return tuple(outs)

        devices = jax.devices()[:NCORES]
        mesh = Mesh(np.asarray(devices), ("core",))
        nio = n_params + len(out_names)
        self.fn = jax.jit(
            shard_map(_body, mesh=mesh, in_specs=(PartitionSpec("core"),) * nio,
                      out_specs=(PartitionSpec("core"),) * len(out_names),
                      check_rep=False),
            donate_argnums=donate, keep_unused=True)

    def __call__(self, in_maps):
        concat_in = [
            np.concatenate([np.asarray(m[nm]) for m in in_maps], axis=0)
            for nm in self.in_names]
        zeros = [np.zeros((NCORES * s[0], *s[1:]), dt)
                 for s, dt in self.zero_shapes]
        out_arrs = self.fn(*concat_in, *zeros)
        return [
            {nm: np.asarray(out_arrs[i]).reshape(
                NCORES, *self.out_avals[i].shape)[c]
             for i, nm in enumerate(self.out_names)}
            for c in range(NCORES)]


class _Res:
    def __init__(self, results):
        self.results = results
        self.exec_time_ns = None
        self.instructions_and_trace = None


def _run_spmd(key, nc, in_maps):
    import time
    if key not in _PROGRAMS or not isinstance(_PROGRAMS.get(key + "_run"), _Runner):
        _PROGRAMS[key + "_run"] = _Runner(nc)
    t0 = time.perf_counter()
    results = _PROGRAMS[key + "_run"](in_maps)
    _LAST_WALL.append(time.perf_counter() - t0)
    return _Res(results)


# ---------------------------------------------------------------- driver
def kernel(x, emb0, emb1, w1, b1, w2, b2, w3, b3, weights_pool, bias_pool):
    x = np.asarray(x, np.float32)
    emb0 = np.asarray(emb0, np.float32)
    emb1 = np.asarray(emb1, np.float32)
    k1, k2 = _programs()
    cores = list(range(NCORES))

    in1 = []
    for c in range(NCORES):
        xs = x[BS * c:BS * (c + 1)]              # (BS, N, C)
        e0 = emb0[BS * c:BS * (c + 1)]           # (BS, N, E)
        in1.append({
            "xr": np.ascontiguousarray(
                xs.reshape(BS, NCH, 128, C).transpose(0, 2, 1, 3)
                .reshape(BS, 128, NCH * C)),
            "xT": np.ascontiguousarray(xs.reshape(BN, C).T),
            "e0T": np.ascontiguousarray(e0.reshape(BN, E).T),
            "w1": np.ascontiguousarray(w1),
            "b1": np.ascontiguousarray(b1.reshape(H, 1)),
            "w2": np.ascontiguousarray(w2),
            "b2": np.ascontiguousarray(b2.reshape(M, 1)),
            "w3": np.ascontiguousarray(w3),
            "b3": np.ascontiguousarray(b3.reshape(E, 1)),
        })
    _LAST_RESULTS.clear()
    _LAST_WALL.clear()
    r1 = _run_spmd("k1", k1, in1)
    _LAST_RESULTS.append(r1)

    z = np.empty((B, N, C), np.float32)
    d = np.empty((B, N), np.float32)
    for c in range(NCORES):
        zT = r1.results[c]["zT"]                 # (BS, C, N)
        dc = r1.results[c]["dcol"]               # (BS, 128, NCH)
        z[BS * c:BS * (c + 1)] = zT.transpose(0, 2, 1)
        d[BS * c:BS * (c + 1)] = dc.transpose(0, 2, 1).reshape(BS, N)

    y = d[:, :, None] * z                        # outer D scaling on host
    xg = np.concatenate([x, y], axis=2)          # (B, N, KI)
    xgT = np.ascontiguousarray(xg.transpose(2, 1, 0))  # (KI, N, B)
    poolT = np.ascontiguousarray(
        weights_pool.reshape(E, KI, O).transpose(0, 2, 1).reshape(E, O * KI))

    in2 = []
    for c in range(NCORES):
        ns = slice(NS * c, NS * (c + 1))
        in2.append({
            "e1T": np.ascontiguousarray(emb1[ns].T),
            "poolT": poolT,
            "bp": np.ascontiguousarray(bias_pool),
            "xgTs": np.ascontiguousarray(xgT[:, ns].reshape(KI, NS * B)),
        })
    r2 = _run_spmd("k2", k2, in2)
    _LAST_RESULTS.append(r2)

    out = np.empty((B, N, O), np.float32)
    for c in range(NCORES):
        oT = r2.results[c]["outT"]               # (O, NS*B)
        out[:, NS * c:NS * (c + 1)] = oT.reshape(O, NS, B).transpose(2, 1, 0)
    return out


# revision 13
# speedup vs baseline: 1.3510x; 1.3510x over previous
"""DGCN hypernetwork GNN kernel for 8x Trainium2 NeuronCores.

Strategy:
  Kernel 1 (data-parallel over batch, 2 samples/core):
    hypernet MLP -> nodevec V^T; per sample: A = V V^T emitted tile-by-tile on
    the PE (2-way row-group packing since E=16), relu+rowsum fused into the
    PSUM->SBUF eviction (vector tensor_scalar / scalar activation, both with
    accum_out), d = rsqrt(rowsum), z = relu(A) @ (d*x).  relu(A) lives only in
    SBUF (16 MB/sample) - never touches HBM.
  Host: y = d*z, assemble x_g^T = [x^T; y^T], reshard by node.
  Kernel 2 (data-parallel over nodes, 256 nodes/core):
    W[n] = sum_d emb1[n,d] pool[d] materialized on PE, block-diagonal
    projection out[:,n,:] = xg[:,n,:] @ W[n] + bias[n].
"""

import numpy as np

# ---------------------------------------------------------------- shapes
B, N, C, E, O = 16, 2048, 64, 16, 64
H, M, K = 16, 2, 2
NCORES = 8
BS = B // NCORES          # samples per core in kernel 1
NS = N // NCORES          # nodes per core in kernel 2
BN = BS * N               # 4096 rows per core in kernel 1
NCH = N // 128            # 16 m-chunks per sample
KI = K * C                # 128


# ------------------------------------------------- walrus drain workaround
def _apply_tile_patch():
    """This walrus build lowers at most ONE sync wait per CTRL instruction;
    Tile's end-of-kernel drain carries several.  Split extras onto Nops."""
    import concourse.mybir as mybir
    from concourse import tile

    if getattr(tile.TileContext, "_drain_split_patched", False):
        return
    orig = tile.TileContext._drain_and_barrier

    def _split_multiwait(nc):
        for f in nc.m.functions:
            for bb in f.blocks:
                newlist = []
                changed = False
                for ins in bb.instructions:
                    si = ins.sync_info
                    if si is not None and si.on_wait and len(si.on_wait) > 1:
                        waits = list(si.on_wait)
                        for w in waits[:-1]:
                            nop = mybir.InstNoOp(
                                name=f"I-{nc.next_id()}", ins=[], outs=[])
                            nop.engine = ins.engine
                            nop.sync_info = mybir.SyncInfo(
                                on_wait=[w], on_update=[])
                            nc.register_instruction(nop)
                            newlist.append(nop)
                        ins.sync_info = mybir.SyncInfo(
                            on_wait=[waits[-1]], on_update=si.on_update)
                        changed = True
                    newlist.append(ins)
                if changed:
                    bb.instructions[:] = newlist

    def patched(self, tick_clock, wait_clock):
        orig(self, tick_clock, wait_clock)
        _split_multiwait(self.nc)

    tile.TileContext._drain_and_barrier = patched
    tile.TileContext._drain_split_patched = True


# ---------------------------------------------------------------- kernel 1
def _build_k1():
    from concourse import bass, tile
    import concourse.mybir as mybir

    dt = mybir.dt
    f32 = dt.float32
    nc = bass.Bass()

    xr = nc.dram_tensor("xr", [BS, 128, NCH * C], f32, kind="ExternalInput").ap()
    xT = nc.dram_tensor("xT", [C, BN], f32, kind="ExternalInput").ap()
    e0T = nc.dram_tensor("e0T", [E, BN], f32, kind="ExternalInput").ap()
    w1 = nc.dram_tensor("w1", [C, H], f32, kind="ExternalInput").ap()
    b1 = nc.dram_tensor("b1", [H, 1], f32, kind="ExternalInput").ap()
    w2 = nc.dram_tensor("w2", [H, M], f32, kind="ExternalInput").ap()
    b2 = nc.dram_tensor("b2", [M, 1], f32, kind="ExternalInput").ap()
    w3 = nc.dram_tensor("w3", [M, E], f32, kind="ExternalInput").ap()
    b3 = nc.dram_tensor("b3", [E, 1], f32, kind="ExternalInput").ap()
    zT_out = nc.dram_tensor("zT", [BS, 128, N // 2], f32, kind="ExternalOutput").ap()
    d_out = nc.dram_tensor("dcol", [BS, 128, NCH], f32, kind="ExternalOutput").ap()

    AF = mybir.ActivationFunctionType
    AL = mybir.AluOpType

    from contextlib import ExitStack
    with tile.TileContext(nc) as tc, ExitStack() as ctx:
        cpool = ctx.enter_context(tc.tile_pool(name="consts", bufs=1))
        w1_s = cpool.tile([C, H], f32, tag="w1")
        nc.sync.dma_start(w1_s[:], w1[:])
        w2_s = cpool.tile([H, M], f32, tag="w2")
        nc.sync.dma_start(w2_s[:], w2[:])
        w3_s = cpool.tile([M, E], f32, tag="w3")
        nc.sync.dma_start(w3_s[:], w3[:])
        b1_s = cpool.tile([H, 1], f32, tag="b1")
        nc.sync.dma_start(b1_s[:], b1[:])
        b2_s = cpool.tile([M, 1], f32, tag="b2")
        nc.sync.dma_start(b2_s[:], b2[:])
        b3_s = cpool.tile([E, 1], f32, tag="b3")
        nc.sync.dma_start(b3_s[:], b3[:])

        big = ctx.enter_context(tc.tile_pool(name="big", bufs=1))
        # relu(A) store for one sample: 16 chunk-rows of [128, 2048]
        Tbig = big.tile([128, NCH * N], f32, tag="Tbig")
        # V^T replicated at partition offsets 0 and 32, one per sample
        vrep = [big.tile([128, N], f32, tag=f"vrep{s}", name=f"vrep{s}") for s in range(BS)]
        # x in [m-chunk partition, (chunk, c)] layout, per sample
        xs = [big.tile([128, NCH * C], f32, tag=f"xs{s}", name=f"xs{s}") for s in range(BS)]
        xp = big.tile([128, NCH * C], f32, tag="xp")
        zTs = big.tile([128, N // 2], f32, tag="zTs")
        acc = big.tile([128, 2 * NCH], f32, tag="acc")
        rcol = big.tile([128, NCH], f32, tag="rcol")
        rinv = big.tile([128, NCH], f32, tag="rinv")
        dcol = big.tile([128, NCH], f32, tag="dcol")

        for s in range(BS):
            nc.sync.dma_start(xs[s][:], xr[s])

        # ---------------- hypernet MLP, feature-on-partition layout -------
        CH = 512
        with tc.tile_pool(name="mlp", bufs=2) as mp, \
             tc.tile_pool(name="mlppsum", bufs=2, space="PSUM") as pp:
            for ch in range(BN // CH):
                s, off = divmod(ch * CH, N)
                xTc = mp.tile([C, CH], f32, tag="xTc")
                nc.sync.dma_start(xTc[:], xT[:, bass.ts(ch, CH)])
                e0c = mp.tile([E, CH], f32, tag="e0c")
                nc.sync.dma_start(e0c[:], e0T[:, bass.ts(ch, CH)])

                p1 = pp.tile([H, CH], f32, tag="p1")
                nc.tensor.matmul(p1[:], lhsT=w1_s[:], rhs=xTc[:],
                                 start=True, stop=True)
                h1 = mp.tile([H, CH], f32, tag="h1")
                nc.scalar.activation(h1[:], p1[:], AF.Sigmoid, bias=b1_s[:])

                p2 = pp.tile([M, CH], f32, tag="p2")
                nc.tensor.matmul(p2[:], lhsT=w2_s[:], rhs=h1[:],
                                 start=True, stop=True)
                h2 = mp.tile([M, CH], f32, tag="h2")
                nc.scalar.activation(h2[:], p2[:], AF.Sigmoid, bias=b2_s[:])

                p3 = pp.tile([E, CH], f32, tag="p3")
                nc.tensor.matmul(p3[:], lhsT=w3_s[:], rhs=h2[:],
                                 start=True, stop=True)
                filt = mp.tile([E, CH], f32, tag="filt")
                nc.scalar.activation(filt[:], p3[:], AF.Identity, bias=b3_s[:])
                # nodevec = tanh(emb0 * filt)
                prod = mp.tile([E, CH], f32, tag="prod")
                nc.vector.tensor_tensor(out=prod[:], in0=filt[:], in1=e0c[:],
                                        op=AL.mult)
                nc.scalar.activation(vrep[s][0:E, off:off + CH], prod[:],
                                     AF.Tanh)
        for s in range(BS):
            for g in (32, 64, 96):
                nc.sync.dma_start(vrep[s][g:g + E, :], vrep[s][0:E, :])

        # ---------------- per-sample adjacency + propagate ----------------
        for s in range(BS):
            # emit A = V V^T in (i, half) units; 4-way row-group packing;
            # relu+rowsum fused on PSUM eviction, alternating engines
            with tc.tile_pool(name=f"pa{s}", bufs=4, space="PSUM") as pa_pool:
                for u in range(2 * NCH):
                    i, h = divmod(u, 2)
                    g = 32 * (u % 4)
                    lhs = vrep[s][g:g + E, bass.ts(i, 128)]
                    pa = pa_pool.tile([128, 1024], f32, tag="pa")
                    for q in range(2):
                        nc.tensor.matmul(
                            pa[:, bass.ts(q, 512)], lhsT=lhs,
                            rhs=vrep[s][g:g + E, bass.ts(2 * h + q, 512)],
                            start=True, stop=True, tile_position=(g, 0))
                    dst = Tbig[:, i * N + 1024 * h:i * N + 1024 * (h + 1)]
                    ac = acc[:, 16 * h + i:16 * h + i + 1]
                    if u % 2 == 0:
                        nc.vector.tensor_scalar(
                            dst, pa[:], 0.0, None,
                            op0=AL.max, op1=AL.add, accum_out=ac)
                    else:
                        nc.scalar.activation(dst, pa[:], AF.Relu, accum_out=ac)

            # d = 1/sqrt(rowsum)
            nc.vector.tensor_tensor(out=rcol[:], in0=acc[:, 0:NCH],
                                    in1=acc[:, NCH:2 * NCH], op=AL.add)
            nc.vector.reciprocal(rinv[:], rcol[:])
            nc.scalar.activation(dcol[:], rinv[:], AF.Sqrt)
            nc.sync.dma_start(d_out[s], dcol[:])

            # x' = d * x   (split across vector/scalar engines)
            for c in range(NCH):
                if c % 2 == 0:
                    nc.vector.tensor_scalar(
                        xp[:, bass.ts(c, C)], xs[s][:, bass.ts(c, C)],
                        dcol[:, c:c + 1], None, op0=AL.mult)
                else:
                    nc.scalar.activation(
                        xp[:, bass.ts(c, C)], xs[s][:, bass.ts(c, C)],
                        AF.Copy, scale=dcol[:, c:c + 1])

            # z^T = (relu(A) @ x')^T ; two col-group chains over n-halves
            with tc.tile_pool(name=f"pz{s}", bufs=1, space="PSUM") as pz_pool:
                pz = pz_pool.tile([128, N // 2], f32, tag="pz")
                for j in range(2):
                    for c in range(NCH):
                        nc.tensor.matmul(
                            pz[0:64, bass.ts(j, 512)],
                            lhsT=xp[:, bass.ts(c, C)],
                            rhs=Tbig[:, c * N + 512 * j:c * N + 512 * (j + 1)],
                            start=(c == 0), stop=(c == NCH - 1),
                            tile_position=(0, 0))
                    for c in range(NCH):
                        nc.tensor.matmul(
                            pz[64:128, bass.ts(j, 512)],
                            lhsT=xp[:, bass.ts(c, C)],
                            rhs=Tbig[:, c * N + 1024 + 512 * j:
                                     c * N + 1024 + 512 * (j + 1)],
                            start=(c == 0), stop=(c == NCH - 1),
                            tile_position=(0, 64))
                nc.vector.tensor_copy(zTs[:, 0:512], pz[:, 0:512])
                nc.scalar.copy(zTs[:, 512:1024], pz[:, 512:1024])
            nc.sync.dma_start(zT_out[s], zTs[:])

    return nc


# ---------------------------------------------------------------- kernel 2
def _build_k2():
    from concourse import bass, tile
    import concourse.mybir as mybir

    dt = mybir.dt
    f32 = dt.float32
    nc = bass.Bass()

    e1T = nc.dram_tensor("e1T", [E, NS], f32, kind="ExternalInput").ap()
    poolT = nc.dram_tensor("poolT", [E, O * KI], f32, kind="ExternalInput").ap()
    bp = nc.dram_tensor("bp", [E, O], f32, kind="ExternalInput").ap()
    xgTs = nc.dram_tensor("xgTs", [KI, NS * B], f32, kind="ExternalInput").ap()
    outT = nc.dram_tensor("outT", [O, NS * B], f32, kind="ExternalOutput").ap()

    AL = mybir.AluOpType

    with tile.TileContext(nc) as tc:
        with tc.tile_pool(name="sb", bufs=1) as sb, \
             tc.tile_pool(name="pw", bufs=3, space="PSUM") as pwp, \
             tc.tile_pool(name="po", bufs=4, space="PSUM") as pop:
            e1T_s = sb.tile([E, NS], f32, tag="e1T")
            nc.sync.dma_start(e1T_s[:], e1T[:])
            pT_s = sb.tile([E, O * KI], f32, tag="pT")
            nc.sync.dma_start(pT_s[:], poolT[:])
            bp_s = sb.tile([E, O], f32, tag="bp")
            nc.sync.dma_start(bp_s[:], bp[:])
            xg_s = sb.tile([KI, NS * B], f32, tag="xg")
            nc.sync.dma_start(xg_s[:], xgTs[:])
            Ws = sb.tile([KI, O * NS], f32, tag="Ws")
            biasT = sb.tile([O, NS], f32, tag="biasT")
            outs = sb.tile([O, NS * B], f32, tag="outs")

            # W[:, o*NS + n] over ki partitions = sum_d emb1[n,d] pool[d,ki,o]
            for o in range(O):
                pw = pwp.tile([KI, NS], f32, tag="pw")
                nc.tensor.matmul(pw[:], lhsT=pT_s[:, bass.ts(o, KI)],
                                 rhs=e1T_s[:], start=True, stop=True)
                if o % 2 == 0:
                    nc.vector.tensor_copy(Ws[:, bass.ts(o, NS)], pw[:])
                else:
                    nc.scalar.copy(Ws[:, bass.ts(o, NS)], pw[:])

            pb = pwp.tile([O, NS], f32, tag="pb", bufs=1)
            nc.tensor.matmul(pb[:], lhsT=bp_s[:], rhs=e1T_s[:],
                             start=True, stop=True)
            nc.vector.tensor_copy(biasT[:], pb[:])

            # per-node projection, 8 nodes per PSUM bank group
            Wv = Ws[:].rearrange("p (o n) -> p o n", o=O, n=NS)
            for grp in range(NS // 8):
                po = pop.tile([O, 8 * B], f32, tag="po")
                for t in range(8):
                    n = 8 * grp + t
                    nc.tensor.matmul(
                        po[:, bass.ts(t, B)], lhsT=Wv[:, :, n:n + 1],
                        rhs=xg_s[:, bass.ts(n, B)], start=True, stop=True)
                bslice = biasT[:, 8 * grp:8 * grp + 8]
                bbc = bslice.unsqueeze(2).broadcast_to([O, 8, B])
                nc.vector.tensor_tensor(
                    out=outs[:, grp * 8 * B:(grp + 1) * 8 * B], in0=po[:],
                    in1=bbc, op=AL.add)
            nc.sync.dma_start(outT[:], outs[:])
    return nc


_PROGRAMS = {}
_LAST_RESULTS = []
_LAST_WALL = []


def _programs():
    if "k1" not in _PROGRAMS:
        _apply_tile_patch()
        _PROGRAMS["k1"] = _build_k1()
        _PROGRAMS["k2"] = _build_k2()
    return _PROGRAMS["k1"], _PROGRAMS["k2"]


class _Runner:
    """Cached jitted SPMD executor (mirrors bass2jax.run_bass_via_pjrt but
    keeps the jit closure alive so repeat calls don't recompile)."""

    def __init__(self, nc):
        import jax
        import concourse.mybir as mybir
        from jax.sharding import Mesh, PartitionSpec
        from jax.experimental.shard_map import shard_map
        from concourse.bass2jax import _bass_exec_p, install_neuronx_cc_hook

        install_neuronx_cc_hook()
        self.nc = nc
        in_names, out_names, out_avals, zero_shapes = [], [], [], []
        for alloc in nc.m.functions[0].allocations:
            if not isinstance(alloc, mybir.MemoryLocationSet):
                continue
            name = alloc.memorylocations[0].name
            if alloc.kind == "ExternalInput":
                in_names.append(name)
            elif alloc.kind == "ExternalOutput":
                out_names.append(name)
                shape = tuple(alloc.tensor_shape)
                dtype = mybir.dt.np(alloc.dtype)
                out_avals.append(jax.core.ShapedArray(shape, dtype))
                zero_shapes.append((shape, dtype))
        self.in_names, self.out_names = in_names, out_names
        self.out_avals, self.zero_shapes = out_avals, zero_shapes
        n_params = len(in_names)
        all_names = tuple(in_names + out_names)
        donate = tuple(range(n_params, n_params + len(out_names)))

        def _body(*args):
            outs = _bass_exec_p.bind(
                *args, out_avals=tuple(out_avals), in_names=all_names,
                out_names=tuple(out_names),
                lowering_input_output_aliases=(),
                sim_require_finite=True, sim_require_nnan=True, nc=nc)
            return tuple(outs)

        devices = jax.devices()[:NCORES]
        mesh = Mesh(np.asarray(devices), ("core",))
        nio = n_params + len(out_names)
        self.fn = jax.jit(
            shard_map(_body, mesh=mesh, in_specs=(PartitionSpec("core"),) * nio,
                      out_specs=(PartitionSpec("core"),) * len(out_names),
                      check_rep=False),
            donate_argnums=donate, keep_unused=True)

    def __call__(self, in_maps):
        concat_in = [
            np.concatenate([np.asarray(m[nm]) for m in in_maps], axis=0)
            for nm in self.in_names]
        zeros = [np.zeros((NCORES * s[0], *s[1:]), dt)
                 for s, dt in self.zero_shapes]
        out_arrs = self.fn(*concat_in, *zeros)
        return [
            {nm: np.asarray(out_arrs[i]).reshape(
                NCORES, *self.out_avals[i].shape)[c]
             for i, nm in enumerate(self.out_names)}
            for c in range(NCORES)]


class _Res:
    def __init__(self, results):
        self.results = results
        self.exec_time_ns = None
        self.instructions_and_trace = None


def _run_spmd(key, nc, in_maps):
    import time
    if key not in _PROGRAMS or not isinstance(_PROGRAMS.get(key + "_run"), _Runner):
        _PROGRAMS[key + "_run"] = _Runner(nc)
    t0 = time.perf_counter()
    results = _PROGRAMS[key + "_run"](in_maps)
    _LAST_WALL.append(time.perf_counter() - t0)
    return _Res(results)


# ---------------------------------------------------------------- driver
def kernel(x, emb0, emb1, w1, b1, w2, b2, w3, b3, weights_pool, bias_pool):
    x = np.asarray(x, np.float32)
    emb0 = np.asarray(emb0, np.float32)
    emb1 = np.asarray(emb1, np.float32)
    k1, k2 = _programs()
    cores = list(range(NCORES))

    in1 = []
    for c in range(NCORES):
        xs = x[BS * c:BS * (c + 1)]              # (BS, N, C)
        e0 = emb0[BS * c:BS * (c + 1)]           # (BS, N, E)
        in1.append({
            "xr": np.ascontiguousarray(
                xs.reshape(BS, NCH, 128, C).transpose(0, 2, 1, 3)
                .reshape(BS, 128, NCH * C)),
            "xT": np.ascontiguousarray(xs.reshape(BN, C).T),
            "e0T": np.ascontiguousarray(e0.reshape(BN, E).T),
            "w1": np.ascontiguousarray(w1),
            "b1": np.ascontiguousarray(b1.reshape(H, 1)),
            "w2": np.ascontiguousarray(w2),
            "b2": np.ascontiguousarray(b2.reshape(M, 1)),
            "w3": np.ascontiguousarray(w3),
            "b3": np.ascontiguousarray(b3.reshape(E, 1)),
        })
    _LAST_RESULTS.clear()
    _LAST_WALL.clear()
    r1 = _run_spmd("k1", k1, in1)
    _LAST_RESULTS.append(r1)

    z = np.empty((B, N, C), np.float32)
    d = np.empty((B, N), np.float32)
    for c in range(NCORES):
        zT = r1.results[c]["zT"]                 # (BS, 128, N//2)
        dc = r1.results[c]["dcol"]               # (BS, 128, NCH)
        z[BS * c:BS * (c + 1)] = (zT.reshape(BS, 2, C, N // 2)
                                  .transpose(0, 1, 3, 2).reshape(BS, N, C))
        d[BS * c:BS * (c + 1)] = dc.transpose(0, 2, 1).reshape(BS, N)

    y = d[:, :, None] * z                        # outer D scaling on host
    xg = np.concatenate([x, y], axis=2)          # (B, N, KI)
    xgT = np.ascontiguousarray(xg.transpose(2, 1, 0))  # (KI, N, B)
    poolT = np.ascontiguousarray(
        weights_pool.reshape(E, KI, O).transpose(0, 2, 1).reshape(E, O * KI))

    in2 = []
    for c in range(NCORES):
        ns = slice(NS * c, NS * (c + 1))
        in2.append({
            "e1T": np.ascontiguousarray(emb1[ns].T),
            "poolT": poolT,
            "bp": np.ascontiguousarray(bias_pool),
            "xgTs": np.ascontiguousarray(xgT[:, ns].reshape(KI, NS * B)),
        })
    r2 = _run_spmd("k2", k2, in2)
    _LAST_RESULTS.append(r2)

    out = np.empty((B, N, O), np.float32)
    for c in range(NCORES):
        oT = r2.results[c]["outT"]               # (O, NS*B)
        out[:, NS * c:NS * (c + 1)] = oT.reshape(O, NS, B).transpose(2, 1, 0)
    return out
